# revision 46
# baseline (speedup 1.0000x reference)
"""Trainium2 Bass kernel for the PrimedGKA layer (gated linear attention with
Chebyshev query refinement), tensor-parallel over the 16 query heads across
8 NeuronCores (2 q-heads + their shared kv-head per core); per-core partial
out-projections are summed on the host.

Restructured vs the baseline to relieve the DVE and PE-sequencer bottlenecks:
feature-major gate projection transposed once via the DMA XBAR, one batched
cumsum matmul for all chunks' decay logs, decay masks via Activation-engine
exp with a per-partition -G_s bias (PE matmul supplies the G_t broadcast and
the -30000 causal clamp), the causal conv as 4 accumulating diagonal matmuls
on PE, raw-k score matmuls with 1/||k|| folded into the query/value sides,
head-stacked 128-partition pass-2 layout so the per-head recurrent-state
matmuls merge into block-diagonal single matmuls, and elementwise traffic
split across DVE / GpSimd / Act.

Self-contained: hardcodes all shapes from the problem spec.
"""
import numpy as np

B, T, D = 1, 1024, 1024
HQ, HKV, HK, HV = 16, 4, 64, 64
KW = 4
NCORES = 8
L = 128                 # chunk length
NCH = T // L            # 8 chunks
CHEB_DAMP = 0.25
EPS = 1e-6

_PROG_CACHE = {}


def _build_program(dbg=False, reps=1):
    import concourse.bacc as bacc
    import concourse.mybir as mybir
    from concourse.tile import TileContext

    dt = mybir.dt
    f32 = dt.float32
    f32r = dt.float32r
    f16 = dt.float16
    AF = mybir.ActivationFunctionType
    ALU = mybir.AluOpType
    X = mybir.AxisListType.X

    nc = bacc.Bacc("TRN2", target_bir_lowering=False, debug=False,
                   num_devices=NCORES)

    xT16 = nc.dram_tensor("xT16", [D, T], f16, kind="ExternalInput")
    wcat = nc.dram_tensor("wcat", [D, 256], f16, kind="ExternalInput")
    wg5 = nc.dram_tensor("wg5", [D, 5], f16, kind="ExternalInput")
    convd = nc.dram_tensor("convd", [128, 8 * 128], f16, kind="ExternalInput")
    wo = nc.dram_tensor("wo", [128, D], f16, kind="ExternalInput")
    alog = nc.dram_tensor("alog", [1, 2], f32, kind="ExternalInput")
    dtbbc = nc.dram_tensor("dtbbc", [128, 2], f32, kind="ExternalInput")
    um = nc.dram_tensor("um", [128, 128], f32, kind="ExternalInput")
    up16 = nc.dram_tensor("up16", [128, 128], f16, kind="ExternalInput")
    rhs30k = nc.dram_tensor("rhs30k", [128, 512], f16, kind="ExternalInput")
    iden = nc.dram_tensor("iden", [128, 128], f16, kind="ExternalInput")
    selall = nc.dram_tensor("selall", [16, 1024], f16, kind="ExternalInput")
    outp = nc.dram_tensor("outp", [T, D], f16, kind="ExternalOutput")
    if dbg:
        dSq = nc.dram_tensor("dSq", [128, T], f16, kind="ExternalOutput")
        dSkv = nc.dram_tensor("dSkv", [128, T], f16, kind="ExternalOutput")
        dgtm = nc.dram_tensor("dgtm", [128, 64], f16, kind="ExternalOutput")
        dgt = nc.dram_tensor("dgt", [128, 2048], f16, kind="ExternalOutput")
        dgb = nc.dram_tensor("dgb", [128, 1024], f16, kind="ExternalOutput")
        dkvtm = nc.dram_tensor("dkvtm", [8, 128, 192], f16, kind="ExternalOutput")
        dhmk = nc.dram_tensor("dhmk", [8, 128, 128], f16, kind="ExternalOutput")
        dhmv = nc.dram_tensor("dhmv", [8, 128, 128], f16, kind="ExternalOutput")
        dqse = nc.dram_tensor("dqse", [128, 2048], f16, kind="ExternalOutput")
        dxce = nc.dram_tensor("dxce", [8, 128, 256], f16, kind="ExternalOutput")
        dsfin = nc.dram_tensor("dsfin", [128, 16], f32, kind="ExternalOutput")
        dinvk = nc.dram_tensor("dinvk", [128, 8], f32, kind="ExternalOutput")

    with TileContext(nc) as tc:
      import contextlib
      for _rep in range(reps):
        ctx = contextlib.ExitStack()
        with ctx:
            pers = ctx.enter_context(tc.tile_pool(name="pers", bufs=1))
            p_sm = ctx.enter_context(tc.tile_pool(name="p_sm", bufs=9))
            p_kv = ctx.enter_context(tc.tile_pool(name="p_kv", bufs=9))
            p_xq = ctx.enter_context(tc.tile_pool(name="p_xq", bufs=10))
            p_out = ctx.enter_context(tc.tile_pool(name="p_out", bufs=6))
            ps_all = ctx.enter_context(tc.tile_pool(name="ps_all", bufs=6, space="PSUM"))
            ps_oy = ctx.enter_context(tc.tile_pool(name="ps_oy", bufs=2, space="PSUM"))

            # ================= persistent loads =================
            wcat_sb = pers.tile([128, 8, 256], f16)
            nc.sync.dma_start(out=wcat_sb[:], in_=wcat[:].rearrange("(a p) c -> p a c", p=128))
            wg5_sb = pers.tile([128, 8, 5], f16)
            nc.sync.dma_start(out=wg5_sb[:], in_=wg5[:].rearrange("(a p) c -> p a c", p=128))
            convd_sb = pers.tile([128, 8, 128], f16)
            nc.scalar.dma_start(out=convd_sb[:], in_=convd[:].rearrange("p (a c) -> p a c", a=8))
            xt16_sb = pers.tile([128, 8, T], f16)
            for d in range(8):
                eng = nc.sync if d % 2 == 0 else nc.scalar
                eng.dma_start(out=xt16_sb[:, d, :], in_=xT16[d * 128:(d + 1) * 128, :])
            wo_sb = pers.tile([128, D], f16)
            nc.scalar.dma_start(out=wo_sb[:], in_=wo[:])
            alog_sb = pers.tile([1, 2], f32)
            nc.sync.dma_start(out=alog_sb[:], in_=alog[:])
            dtb_sb = pers.tile([128, 2], f32)
            nc.sync.dma_start(out=dtb_sb[:], in_=dtbbc[:])
            um_sb = pers.tile([128, 128], f32)
            nc.sync.dma_start(out=um_sb[:], in_=um[:])
            up16_sb = pers.tile([128, 128], f16)
            nc.scalar.dma_start(out=up16_sb[:], in_=up16[:])
            rhs30k_sb = pers.tile([128, 512], f16)
            nc.scalar.dma_start(out=rhs30k_sb[:], in_=rhs30k[:])
            iden16_sb = pers.tile([128, 128], f16)
            nc.sync.dma_start(out=iden16_sb[:], in_=iden[:])
            selall_sb = pers.tile([16, 1024], f16)
            nc.scalar.dma_start(out=selall_sb[:], in_=selall[:])

            ones128h = pers.tile([1, 128], f16)
            nc.vector.memset(ones128h[:], 1.0)

            # persistent work tiles
            Rq = pers.tile([128, 3 + T], f16)     # padded raw q proj, fm
            Rkv = pers.tile([128, 3 + T], f16)    # padded raw k|v proj, fm
            nc.vector.memset(Rq[:, 0:3], 0.0)
            nc.vector.memset(Rkv[:, 0:3], 0.0)
            Sq = pers.tile([128, T], f16)         # silu(conv(q)), fm (h-stacked)
            Skv = pers.tile([128, T], f16)        # silu(conv(k|v)), fm
            kst = pers.tile([128, T], f16)        # [k ; k] duplicated rows
            gF = pers.tile([5, T], f16)           # feature-major gates
            g_tmT = pers.tile([128, NCH * 8], f16)    # time-major gates (c)(8)
            e_a = pers.tile([128, 16], f32)
            sp_a = pers.tile([128, 16], f32)
            g_all = pers.tile([128, 16], f32)
            e_ab = pers.tile([128, 24], f32)
            d_ab = pers.tile([128, 24], f32)
            ab_all = pers.tile([128, 24], f32)    # (c)(al0, al1, beta) sigmoids
            asq = pers.tile([128, 16], f32)       # alpha^2 (c)(h)
            negG = pers.tile([128, 16], f32)
            growall = pers.tile([16, 128], f32)
            growhi = pers.tile([16, 128], f16)
            growres = pers.tile([16, 128], f16)
            growrowh = pers.tile([1, 2048], f16)
            growrowr = pers.tile([1, 2048], f16)
            gamlog16 = pers.tile([16, 128], f16)
            gt_all = pers.tile([128, 2048], f16)  # decay masks (c)(h)(t)
            gb_st = pers.tile([128, 1024], f16)   # gamma, head-stacked (c)(t)
            sqk_all = pers.tile([128, 512], f16)
            ssk = pers.tile([128, 8], f32)
            lnk = pers.tile([128, 8], f32)
            invk = pers.tile([128, 8], f32)
            sq_all = pers.tile([128, 1024], f16)  # rms squares (c)(h*64)
            sso = pers.tile([128, 16], f32)
            uvar = pers.tile([128, 16], f32)
            lno = pers.tile([128, 16], f32)
            sfac = pers.tile([128, 16], f32)
            sfin = pers.tile([128, 16], f32)
            era = pers.tile([1, 2], f16)
            negea = pers.tile([128, 2], f32)
            glf = pers.tile([128, 8], f32)        # gammaL per chunk, h-stacked
            b1em12 = pers.tile([128, 1], f32)
            nc.vector.memset(b1em12[:], 1e-12)
            bepsk = pers.tile([128, 1], f32)
            nc.vector.memset(bepsk[:], EPS * HK)

            # q in block-diagonal stacked layout: rows 0:64 head0 (cols c*256
            # .. +128), rows 64:128 head1 (cols c*256+128 .. +256), 0 elsewhere
            qse = pers.tile([128, 2048], f16)
            nc.vector.memset(qse[0:64, :].rearrange("p (c a t) -> p c a t", c=NCH, a=2)[:, :, 1, :], 0.0)
            nc.vector.memset(qse[64:128, :].rearrange("p (c a t) -> p c a t", c=NCH, a=2)[:, :, 0, :], 0.0)

            kvt = [pers.tile([128, 128], f16, name=f"kvt{c}") for c in range(NCH)]
            kvtm = [pers.tile([128, 192], f16, name=f"kvtm{c}") for c in range(NCH)]
            kw = [pers.tile([128, 128], f16, name=f"kw{c}") for c in range(NCH)]
            knegp = [pers.tile([128, 256], f16, name=f"knegp{c}") for c in range(NCH)]
            hmk = [pers.tile([128, 128], f16, name=f"hmk{c}") for c in range(NCH)]
            hmv = [pers.tile([128, 128], f16, name=f"hmv{c}") for c in range(NCH)]
            xceall = pers.tile([128, 2048], f16)
            xv0 = xceall[0:64, :].rearrange("p (c a t) -> p c a t", c=NCH, a=2)
            xv1 = xceall[64:128, :].rearrange("p (c a t) -> p c a t", c=NCH, a=2)
            nc.vector.memset(xv0[:, :, 1, :], 0.0)
            nc.vector.memset(xv1[:, :, 0, :], 0.0)
            for c in range(NCH):
                # zero-pad: knegp halves, hm block-diag off-blocks, xce quadrants
                nc.gpsimd.memset(knegp[c][:, 64:192], 0.0)
                if c > 0:
                    nc.gpsimd.memset(hmk[c][0:64, 64:128], 0.0)
                    nc.gpsimd.memset(hmk[c][64:128, 0:64], 0.0)
                    nc.gpsimd.memset(hmv[c][0:64, 64:128], 0.0)
                    nc.gpsimd.memset(hmv[c][64:128, 0:64], 0.0)


            # ============ front: projections / gates / conv, interleaved ====
            # ordering tuned so each engine queue receives work in expected
            # readiness order (in-order queues suffer head-of-line blocking)
            proj_ps = {}

            def proj_mms(ct, dlist):
                c0 = ct * 128
                if ct not in proj_ps:
                    proj_ps[ct] = [ps_all.tile([128, 512], f32, tag="ps",
                                               name=f"pj{ct}{th}") for th in range(2)]
                for d in dlist:
                    for th in range(2):
                        nc.tensor.matmul(proj_ps[ct][th][:], wcat_sb[:, d, c0:c0 + 128],
                                         xt16_sb[:, d, th * 512:(th + 1) * 512],
                                         start=(d == 0), stop=(d == 7))

            def proj_copies(ct, R):
                for th in range(2):
                    dst = R[:, 3 + th * 512: 3 + (th + 1) * 512]
                    nc.vector.tensor_copy(dst, proj_ps[ct][th][:])

            def conv_mms(tile_idx, R):
                cps = [ps_all.tile([128, 512], f32, tag="ps", name=f"cv{tile_idx}{th}")
                       for th in range(2)]
                for i in range(KW):
                    for th in range(2):
                        nc.tensor.matmul(cps[th][:],
                                         convd_sb[:, tile_idx * 4 + i, :],
                                         R[:, i + th * 512: i + th * 512 + 512],
                                         start=(i == 0), stop=(i == KW - 1))
                return cps

            # -- kv path first: projection, conv, silu, transposes --
            proj_mms(1, range(0, 8))
            proj_copies(1, Rkv)
            cps_kv = conv_mms(1, Rkv)
            for th in range(2):
                nc.scalar.activation(Skv[:, th * 512:(th + 1) * 512], cps_kv[th][:], AF.Silu)
            nc.sync.dma_start(out=kst[0:64, :], in_=Skv[0:64, :])
            nc.scalar.dma_start(out=kst[64:128, :], in_=Skv[0:64, :])
            for c in range(NCH):
                eng = nc.sync if c % 2 == 0 else nc.scalar
                eng.dma_start(out=kvt[c][:], in_=Skv[:, c * L:(c + 1) * L],
                              transpose=True)

            # -- gate projection + transposes --
            pg = [ps_all.tile([5, 512], f32, tag="ps", name=f"pg{th}") for th in range(2)]
            for d in range(8):
                for th in range(2):
                    nc.tensor.matmul(pg[th][:], wg5_sb[:, d, :],
                                     xt16_sb[:, d, th * 512:(th + 1) * 512],
                                     start=(d == 0), stop=(d == 7))
            for th in range(2):
                nc.scalar.copy(gF[:, th * 512:(th + 1) * 512], pg[th][:])
            ps_gt = ps_all.tile([128, 64], f16, tag="ps", name="psgt")
            for c in range(NCH):
                nc.tensor.transpose(ps_gt[:, c * 8:c * 8 + 5],
                                    gF[:, c * 128:(c + 1) * 128],
                                    iden16_sb[0:5, 0:5])
            nc.vector.tensor_copy(g_tmT[:], ps_gt[:])

            # -- gate chain (Act/DVE) --
            gv = g_tmT[:].rearrange("p (c r) -> p c r", r=8)
            eav = e_a[:].rearrange("p (c h) -> p c h", h=2)
            for h in range(2):
                nc.scalar.activation(eav[:, :, h], gv[:, :, h], AF.Exp,
                                     bias=dtb_sb[:, h:h + 1])
            nc.scalar.activation(sp_a[:], e_a[:], AF.Ln, bias=1.0)
            nc.scalar.activation(era[:], alog_sb[:], AF.Exp)
            ps_bc = ps_all.tile([128, 128], f32, tag="ps", name="psbc")
            nc.tensor.matmul(ps_bc[:, 0:2], ones128h[:], era[:], start=True, stop=True)
            nc.vector.tensor_scalar(negea[:], ps_bc[:, 0:2], -1.0, None, ALU.mult)
            spv = sp_a[:].rearrange("p (c h) -> p c h", h=2)
            gav = g_all[:].rearrange("p (c h) -> p c h", h=2)
            for h in range(2):
                nc.vector.tensor_scalar(gav[:, :, h], spv[:, :, h],
                                        negea[:, h:h + 1], None, ALU.mult)
            nc.scalar.activation(e_ab[:].rearrange("p (c r) -> p c r", r=3),
                                 gv[:, :, 2:5], AF.Exp, scale=-1.0)
            nc.vector.tensor_scalar(d_ab[:], e_ab[:], 1.0, None, ALU.add)
            nc.vector.reciprocal(ab_all[:], d_ab[:])
            nc.scalar.activation(asq[:].rearrange("p (c h) -> p c h", h=2),
                                 ab_all[:].rearrange("p (c r) -> p c r", r=3)[:, :, 0:2],
                                 AF.Square)

            # -- k-norm squares (DVE; Act Ln/Exp deferred past the silus) --
            for c in range(NCH):
                nc.vector.tensor_tensor(sqk_all[:, c * 64:(c + 1) * 64],
                                        kvt[c][:, 0:64], kvt[c][:, 0:64], ALU.mult)

            # -- q path: q-proj + q-conv + silus early; masks after --
            proj_mms(0, range(0, 4))
            psG = ps_all.tile([128, 16], f32, tag="ps", name="psG")
            nc.tensor.matmul(psG[:], um_sb[:], g_all[:], start=True, stop=True)
            nc.vector.tensor_scalar(negG[:], psG[:], -1.0, None, ALU.mult)
            psGr = ps_all.tile([16, 128], f32, tag="ps", name="psGr")
            nc.tensor.matmul(psGr[:], g_all[:], um_sb[:], start=True, stop=True)
            nc.vector.tensor_copy(growall[:], psGr[:])
            nc.vector.tensor_copy(growhi[:], psGr[:])
            nc.vector.tensor_tensor(growres[:], growall[:], growhi[:], ALU.subtract)
            nc.sync.dma_start(out=growrowh[:], in_=growhi[:])
            nc.scalar.dma_start(out=growrowr[:], in_=growres[:])
            nc.vector.tensor_copy(gamlog16[:], psGr[:])
            proj_mms(0, range(4, 8))
            proj_copies(0, Rq)
            cps_q = conv_mms(0, Rq)
            for th in range(2):
                nc.scalar.activation(Sq[:, th * 512:(th + 1) * 512], cps_q[th][:], AF.Silu)

            # k-norm invk (Act Ln/Exp, after the silu block in queue order)
            for c in range(1, NCH, 2):
                nc.vector.tensor_reduce(
                    ssk[:, c - 1:c + 1],
                    sqk_all[:, (c - 1) * 64:(c + 1) * 64]
                    .rearrange("p (c k) -> p c k", c=2), X, ALU.add)
                nc.scalar.activation(lnk[:, c - 1:c + 1], ssk[:, c - 1:c + 1],
                                     AF.Ln, bias=b1em12[:])
                nc.scalar.activation(invk[:, c - 1:c + 1], lnk[:, c - 1:c + 1],
                                     AF.Exp, scale=-0.5)

            # gamma broadcast (log-domain selector matmul, exp)
            for b in range(2):
                psgb = ps_all.tile([128, 512], f32, tag="ps", name=f"psgb{b}")
                for k in range(4):
                    c = b * 4 + k
                    nc.tensor.matmul(psgb[:, k * 128:(k + 1) * 128],
                                     selall_sb[:, c * 128:(c + 1) * 128],
                                     gamlog16[:], start=True, stop=True)
                for k in range(2):
                    nc.scalar.activation(gb_st[:, b * 512 + k * 256: b * 512 + (k + 1) * 256],
                                         psgb[:, k * 256:(k + 1) * 256], AF.Exp)
            nc.vector.tensor_copy(
                glf[:], gb_st[:].rearrange("p (c t) -> p c t", c=NCH)[:, :, L - 1])

            # decay masks
            for b in range(4):
                psGB = ps_all.tile([128, 512], f32, tag="ps", name=f"psGB{b}")
                nc.tensor.matmul(psGB[:], ones128h[:], growrowh[:, b * 512:(b + 1) * 512],
                                 start=True, stop=False)
                nc.tensor.matmul(psGB[:], ones128h[:], growrowr[:, b * 512:(b + 1) * 512],
                                 start=False, stop=False)
                nc.tensor.matmul(psGB[:], up16_sb[:], rhs30k_sb[:],
                                 start=False, stop=True)
                for k in range(4):
                    r = b * 4 + k
                    nc.scalar.activation(gt_all[:, r * 128:(r + 1) * 128],
                                         psGB[:, k * 128:(k + 1) * 128], AF.Exp,
                                         bias=negG[:, r:r + 1])

            # ============ kvtm + kw + state chain (per chunk) ======
            hm_prev = None
            for c in range(NCH):
                km = kvtm[c]
                nc.vector.tensor_scalar(km[:, 0:64], kvt[c][:, 0:64],
                                        invk[:, c:c + 1], None, ALU.mult)
                nc.vector.tensor_scalar(km[:, 64:128], kvt[c][:, 64:128],
                                        ab_all[:, c * 3 + 2:c * 3 + 3], None, ALU.mult)
                nc.vector.tensor_scalar(km[:, 128:192], km[:, 64:128],
                                        invk[:, c:c + 1], None, ALU.mult)
                nc.gpsimd.tensor_scalar(knegp[c][:, 0:64], km[:, 0:64],
                                        invk[:, c:c + 1], -CHEB_DAMP, ALU.mult, ALU.mult)
                nc.gpsimd.tensor_copy(knegp[c][:, 192:256], knegp[c][:, 0:64])
                for h in range(2):
                    nc.vector.tensor_tensor(
                        kw[c][:, h * 64:(h + 1) * 64], km[:, 0:64],
                        gt_all[:, c * 256 + h * 128 + L - 1: c * 256 + h * 128 + L]
                        .broadcast_to([128, 64]), ALU.mult)
                ps_hm = ps_all.tile([128, 128], f32, tag="ps", name=f"pshm{c}")
                nc.tensor.matmul(ps_hm[:], kw[c][:], kvtm[c][:, 0:128],
                                 start=True, stop=(c == 0))
                if c > 0:
                    dg = p_sm.tile([128, 128], f16, tag="diag", name=f"diag{c}")
                    nc.vector.tensor_scalar(dg[:], iden16_sb[:],
                                            glf[:, c:c + 1], None, ALU.mult)
                    nc.tensor.matmul(ps_hm[:], dg[:], hm_prev[:],
                                     start=False, stop=True)
                if c + 1 < NCH:
                    hm = p_kv.tile([128, 128], f16, tag="hm", name=f"hm{c}")
                    nc.vector.tensor_copy(hm[:], ps_hm[:])
                    hm_prev = hm
                    cn = c + 1
                    nc.vector.tensor_scalar(hmk[cn][0:64, 0:64], ps_hm[0:64, 0:64],
                                            -CHEB_DAMP, None, ALU.mult)
                    nc.vector.tensor_scalar(hmk[cn][64:128, 64:128], ps_hm[64:128, 0:64],
                                            -CHEB_DAMP, None, ALU.mult)
                    nc.vector.tensor_copy(hmv[cn][0:64, 0:64], ps_hm[0:64, 64:128])
                    nc.vector.tensor_copy(hmv[cn][64:128, 64:128], ps_hm[64:128, 64:128])

            if dbg and _rep == 0:
                nc.sync.dma_start(out=dSq[:], in_=Sq[:])
                nc.sync.dma_start(out=dSkv[:], in_=Skv[:])
                nc.sync.dma_start(out=dgtm[:], in_=g_tmT[:])
                nc.sync.dma_start(out=dgt[:], in_=gt_all[:])
                nc.sync.dma_start(out=dgb[:], in_=gb_st[:])
                nc.sync.dma_start(out=dinvk[:], in_=invk[:])
                for c in range(NCH):
                    nc.sync.dma_start(out=dkvtm[c], in_=kvtm[c][:])
                    if c > 0:
                        nc.sync.dma_start(out=dhmk[c], in_=hmk[c][:])
                        nc.sync.dma_start(out=dhmv[c], in_=hmv[c][:])

            # stacked q into block-diag quadrants (feeds pass-2 scores)
            qv = qse[:].rearrange("p (c a t) -> p c a t", c=NCH, a=2)
            nc.vector.tensor_copy(
                qv[0:64, :, 0, :],
                Sq[0:64, :].rearrange("p (c t) -> p c t", c=NCH))
            nc.vector.tensor_copy(
                qv[64:128, :, 1, :],
                Sq[64:128, :].rearrange("p (c t) -> p c t", c=NCH))

            # ============ pass 2: operator applications ============
            grp = list(range(NCH))
            xcur = {ci: qse[:, ci * 256:(ci + 1) * 256] for ci in grp}
            asb = {}
            ps_o = {}
            xgs0 = {}
            for ci in grp:
                if ci > 0:
                    xs = p_xq.tile([128, 128], f16, tag="xgs", name=f"xgs0{ci}")
                    nc.vector.tensor_tensor(
                        xs[0:64, :], qse[0:64, ci * 256:ci * 256 + 128],
                        gb_st[0:64, ci * 128:(ci + 1) * 128], ALU.mult)
                    nc.gpsimd.tensor_tensor(
                        xs[64:128, :], qse[64:128, ci * 256 + 128:(ci + 1) * 256],
                        gb_st[64:128, ci * 128:(ci + 1) * 128], ALU.mult)
                    xgs0[ci] = xs
            for it in range(3):
                ps_p = {}
                pspb_by = {}
                for ci in grp:
                    if ci % 2 == 0:
                        pspb = ps_all.tile([128, 512], f32, tag="ps", name=f"pspb{ci}")
                    pspb_by[ci] = pspb
                    ps_p[ci] = pspb[:, (ci % 2) * 256:(ci % 2 + 1) * 256]
                    nc.tensor.matmul(ps_p[ci], kst[:, ci * L:(ci + 1) * L],
                                     xcur[ci], start=True, stop=True)
                for ci in grp:
                    if ci % 2 == 1:
                        scr = p_kv.tile([128, 512], f16, tag="scr", name=f"scr{ci}")
                        nc.scalar.copy(scr[:], pspb_by[ci][:])
                        a2 = p_kv.tile([128, 512], f16, tag="a", name=f"asb{ci}")
                        nc.vector.tensor_tensor(
                            a2[:], scr[:],
                            gt_all[:, (ci - 1) * 256:(ci + 1) * 256], ALU.mult)
                        asb[ci - 1] = a2[:, 0:256]
                        asb[ci] = a2[:, 256:512]
                if it == 0:
                    xgs = xgs0
                else:
                    xgs = {}
                    for ci in grp:
                        if ci > 0:
                            xs = p_xq.tile([128, 128], f16, tag="xgs", name=f"xgs{it}{ci}")
                            nc.vector.tensor_tensor(
                                xs[0:64, :], xcur[ci][0:64, 0:128],
                                gb_st[0:64, ci * 128:(ci + 1) * 128], ALU.mult)
                            nc.gpsimd.tensor_tensor(
                                xs[64:128, :], xcur[ci][64:128, 128:256],
                                gb_st[64:128, ci * 128:(ci + 1) * 128], ALU.mult)
                            xgs[ci] = xs
                if it < 2:
                    ps_y = {}
                    for ci in grp:
                        if ci % 4 == 0:
                            psyb = ps_oy.tile([128, 512], f32, tag="psy", name=f"psyb{it}{ci}")
                        ps_y[ci] = psyb[:, (ci % 4) * 128:(ci % 4 + 1) * 128]
                        nc.tensor.matmul(ps_y[ci], knegp[ci][:, 0:128], asb[ci][:, 0:128],
                                         start=True, stop=False)
                        nc.tensor.matmul(ps_y[ci], knegp[ci][:, 128:256], asb[ci][:, 128:256],
                                         start=False, stop=(ci == 0))
                        if ci > 0:
                            nc.tensor.matmul(ps_y[ci], hmk[ci][:], xgs[ci][:],
                                             start=False, stop=True)
                        if ci % 4 == 3:
                            p0 = ci - 3
                            scry = p_xq.tile([128, 512], f16, tag="scry", name=f"scry{it}{ci}")
                            nc.scalar.copy(scry[:], psyb[:])
                            for cj in range(p0, p0 + 4):
                                rg = (cj % 4) * 128
                                nc.vector.tensor_tensor(
                                    xceall[0:64, cj * 256:cj * 256 + 128],
                                    scry[0:64, rg:rg + 128],
                                    qse[0:64, cj * 256:cj * 256 + 128], ALU.add)
                                nc.vector.tensor_tensor(
                                    xceall[64:128, cj * 256 + 128:(cj + 1) * 256],
                                    scry[64:128, rg:rg + 128],
                                    qse[64:128, cj * 256 + 128:(cj + 1) * 256], ALU.add)
                                xcur[cj] = xceall[:, cj * 256:(cj + 1) * 256]
                else:
                    for ci in grp:
                        if ci % 4 == 0:
                            psob = ps_oy.tile([128, 512], f32, tag="psy", name=f"psob{ci}")
                        ps_o[ci] = psob[:, (ci % 4) * 128:(ci % 4 + 1) * 128]
                        for h in range(2):
                            nc.tensor.matmul(ps_o[ci][:, h * 64:(h + 1) * 64],
                                             asb[ci][:, h * 128:(h + 1) * 128],
                                             kvtm[ci][:, 128:192],
                                             start=True, stop=(ci == 0))
                            if ci > 0:
                                nc.tensor.matmul(ps_o[ci][:, h * 64:(h + 1) * 64],
                                                 xgs[ci][:],
                                                 hmv[ci][:, h * 64:(h + 1) * 64],
                                                 start=False, stop=True)

            # ---- rmsnorm + alpha gate + out projection, in two halves ----
            out_rr = 0
            for half in range(2):
                hgrp = list(range(half * 4, half * 4 + 4))
                for ci in hgrp:
                    nc.scalar.activation(sq_all[:, ci * 128:(ci + 1) * 128],
                                         ps_o[ci], AF.Square)
                g0 = half * 4
                sl8 = slice(g0 * 2, g0 * 2 + 8)
                nc.vector.tensor_reduce(
                    sso[:, sl8],
                    sq_all[:, g0 * 128:(g0 + 4) * 128]
                    .rearrange("p (r k) -> p r k", k=64), X, ALU.add)
                nc.vector.tensor_tensor(uvar[:, sl8], sso[:, sl8], asq[:, sl8], ALU.mult)
                nc.scalar.activation(lno[:, sl8], uvar[:, sl8], AF.Ln,
                                     bias=bepsk[:], scale=1.0 / HV)
                nc.scalar.activation(sfac[:, sl8], lno[:, sl8], AF.Exp, scale=-0.5)
                nc.vector.tensor_tensor(
                    sfin[:].rearrange("p (c h) -> p c h", h=2)[:, g0:g0 + 4, :],
                    sfac[:].rearrange("p (c h) -> p c h", h=2)[:, g0:g0 + 4, :],
                    ab_all[:].rearrange("p (c r) -> p c r", r=3)[:, g0:g0 + 4, 0:2],
                    ALU.mult)
                for ci in hgrp:
                    o = p_out.tile([128, 128], f16, tag="on", name=f"on{ci}")
                    nc.vector.tensor_tensor(
                        o[:].rearrange("p (h v) -> p h v", h=2), 
                        ps_o[ci][:].rearrange("p (h v) -> p h v", h=2),
                        sfin[:, ci * 2:ci * 2 + 2].unsqueeze(2).broadcast_to([128, 2, 64]),
                        ALU.mult)
                    ps_of = ps_all.tile([128, 128], f16, tag="ps", name=f"psof{ci}")
                    nc.tensor.transpose(ps_of[:], o[:], iden16_sb[:])
                    f = p_out.tile([128, 128], f16, tag="ofm", name=f"ofm{ci}")
                    nc.vector.tensor_copy(f[:], ps_of[:])
                    out_sb = p_out.tile([128, D], f16, tag="outsb")
                    for nh in range(2):
                        ps_out = ps_all.tile([128, 512], f32, tag="ps", name=f"pso2{ci}{nh}")
                        nc.tensor.matmul(ps_out[:], f[:],
                                         wo_sb[:, nh * 512:(nh + 1) * 512],
                                         start=True, stop=True)
                        dst = out_sb[:, nh * 512:(nh + 1) * 512]
                        nc.scalar.copy(dst, ps_out[:])
                    nc.sync.dma_start(out=outp[ci * L:(ci + 1) * L, 0:512],
                                      in_=out_sb[:, 0:512])
                    nc.scalar.dma_start(out=outp[ci * L:(ci + 1) * L, 512:1024],
                                        in_=out_sb[:, 512:1024])

    # Activation-table placement: map Exp/Ln/Square/Copy into the combined
    # natural_log_exp table (hoisted first) so only Silu forces a switch.
    import concourse.bacc as bacc_mod
    from concourse.hw_specs import get_activation_tables as _gat
    orig_tables = _gat(nc.m.arch)
    orig_names = list(orig_tables.keys())
    pref = "natural_log_exp_and_others"
    reordered = {pref: orig_tables[pref],
                 **{k: v for k, v in orig_tables.items() if k != pref}}
    pnames = list(reordered.keys())
    bacc_mod.get_activation_tables = lambda arch: reordered
    try:
        nc.compile()
    finally:
        bacc_mod.get_activation_tables = _gat
    for b in nc.main_func.blocks:
        for i in b.instructions:
            if isinstance(i, mybir.InstLoadActFuncSet):
                i.act_func_set_id = orig_names.index(pnames[i.act_func_set_id])
    return nc


def _prep_core_inputs(c, x, Wq, Wk, Wv, Wconv, Wa, Walpha, Wb, A_log, dt_bias,
                      norm_w, Wo, xT16, iden16, um32, up16, rhs30k, selall):
    f32, f16 = np.float32, np.float16
    h0, h1, hk = 2 * c, 2 * c + 1, c // 2
    wcat = np.hstack([
        Wq[:, h0 * HK:(h0 + 1) * HK], Wq[:, h1 * HK:(h1 + 1) * HK],
        Wk[:, hk * HK:(hk + 1) * HK], Wv[:, hk * HV:(hk + 1) * HV],
    ]).astype(f16)
    wg5 = np.hstack([
        Wa[:, h0:h0 + 1], Wa[:, h1:h1 + 1],
        Walpha[:, h0:h0 + 1], Walpha[:, h1:h1 + 1],
        Wb[:, hk:hk + 1],
    ]).astype(f16)
    qoff, koff, voff = 0, HQ * HK, HQ * HK + HKV * HK
    wcv = np.vstack([
        Wconv[qoff + h0 * HK: qoff + (h0 + 1) * HK],
        Wconv[qoff + h1 * HK: qoff + (h1 + 1) * HK],
        Wconv[koff + hk * HK: koff + (hk + 1) * HK],
        Wconv[voff + hk * HV: voff + (hk + 1) * HV],
    ]).astype(f32)  # [256, 4]: rows = [q(128) | k(64) | v(64)]
    convd = np.zeros((128, 8, 128), f32)
    for tile in range(2):
        ch = wcv[tile * 128:(tile + 1) * 128]   # [128, KW]
        for tap in range(KW):
            np.fill_diagonal(convd[:, tile * 4 + tap, :], ch[:, tap])
    convd = np.ascontiguousarray(convd.reshape(128, 8 * 128)).astype(f16)
    wo_scale = np.tile(np.asarray(norm_w, f32), HQ)
    Wo_s = np.asarray(Wo, f32) * wo_scale[:, None]
    wo = np.ascontiguousarray(
        np.vstack([Wo_s[h0 * HV:(h0 + 1) * HV], Wo_s[h1 * HV:(h1 + 1) * HV]])).astype(f16)
    alog = np.asarray(A_log, f32)[[h0, h1]].reshape(1, 2).copy()
    dtbbc = np.tile(np.asarray(dt_bias, f32)[[h0, h1]].reshape(1, 2), (128, 1)).copy()
    return dict(xT16=xT16, wcat=np.ascontiguousarray(wcat), wg5=wg5,
                convd=convd, wo=wo, alog=alog, dtbbc=dtbbc,
                um=um32, up16=up16, rhs30k=rhs30k, iden=iden16, selall=selall)


def make_in_maps(x, Wq, Wk, Wv, Wconv, Wa, Walpha, Wb, A_log, dt_bias, norm_w, Wo):
    f32, f16 = np.float32, np.float16
    x2 = np.asarray(x, f32).reshape(T, D)
    xT16 = np.ascontiguousarray(x2.T).astype(f16)
    iden16 = np.eye(128, dtype=f16)
    um32 = np.ascontiguousarray(np.triu(np.ones((128, 128), f32)))
    up16 = np.ascontiguousarray(np.triu(np.ones((128, 128), f16), 1))
    rhs30k = np.ascontiguousarray(
        np.hstack([-30000.0 * np.eye(128)] * 4)).astype(f16)
    selall = np.zeros((16, 8, 128), f16)
    for c in range(8):
        selall[c * 2, c, 0:64] = 1.0
        selall[c * 2 + 1, c, 64:128] = 1.0
    selall = np.ascontiguousarray(selall.reshape(16, 1024))
    args = (x, np.asarray(Wq, f32), np.asarray(Wk, f32), np.asarray(Wv, f32),
            np.asarray(Wconv, f32), np.asarray(Wa, f32), np.asarray(Walpha, f32),
            np.asarray(Wb, f32), A_log, dt_bias, norm_w, Wo)
    return [_prep_core_inputs(c, *args, xT16=xT16, iden16=iden16, um32=um32,
                              up16=up16, rhs30k=rhs30k, selall=selall)
            for c in range(NCORES)]


def get_program(dbg=False, reps=1):
    key = (dbg, reps)
    if key not in _PROG_CACHE:
        _PROG_CACHE[key] = _build_program(dbg, reps)
    return _PROG_CACHE[key]


def kernel(**inputs) -> np.ndarray:
    from concourse.bass_utils import run_bass_kernel_spmd
    nc = get_program(dbg=False)
    in_maps = make_in_maps(**inputs)
    res = run_bass_kernel_spmd(nc, in_maps, list(range(NCORES)))
    out = np.zeros((T, D), np.float32)
    for c in range(NCORES):
        out += res.results[c]["outp"].astype(np.float32)
    return out.reshape(B, T, D)
